# revision 8
# baseline (speedup 1.0000x reference)
"""Multi-head self-attention (B=4, T=2048, D=1024, H=16) on 8 TRN2 NeuronCores.

Sharding: core c = 2*b + j computes batch b, heads j*8..j*8+7 (tensor-parallel
over heads), and a partial projection over its 512 attention-output columns.
The host sums the two partial projections per batch. No collectives.

Per-core dataflow (all matmul inputs bf16, fp32 PSUM accumulation):
  - QK^T projection in transposed layout: psum[feat, t] = wqkT.T @ xT
  - V in natural layout [t, vfeat], stored with a ones column per head
    (V_aug[:, 64] = 1) so the attn@V matmul also produces the softmax
    denominator (row 64 of the output).
  - Transposed scores per head: s^T[k_t, q_t] = K^T_tile.T @ Q^T, exp via
    ScalarE (scale=1/8 folded in, no max subtraction: |s|*scale <~ 3).
    Heads are processed in even/odd pairs living on partitions 0:64 / 64:128
    so the K=64 matmuls pack into the PE array's row groups.
  - attn@V: out[65, q_t] = V_aug.T @ exp_s^T accumulated over k tiles;
    rows 0:64 are unnormalized head outputs, row 64 the denominator.
  - Normalize (DVE reciprocal + GpSimd partition broadcast + DVE multiply)
    into A^T[d, t] (bf16), then partial projection y^T = wpT.T @ A^T (fp32).
"""

import os

import numpy as np
import ml_dtypes

import concourse.mybir as mybir
from concourse import bacc
from concourse.tile import TileContext
from concourse.bass_utils import run_bass_kernel_spmd

B, T, D, H = 4, 2048, 1024, 16
HD = D // H
SCALE = HD**-0.5
P = 128
BF = mybir.dt.bfloat16
F32 = mybir.dt.float32
NBF = ml_dtypes.bfloat16

LAST_RESULT = None
_built = None


def _build():
    nc = bacc.Bacc("TRN2", target_bir_lowering=False, debug=False, num_devices=8)

    xT = nc.dram_tensor("xT", [D, T], BF, kind="ExternalInput")  # x[b].T
    wqkT = nc.dram_tensor("wqkT", [D, 1024], BF, kind="ExternalInput")  # (q|k).T shard
    wvT = nc.dram_tensor("wvT", [D, 512], BF, kind="ExternalInput")
    wpT = nc.dram_tensor("wpT", [512, D], BF, kind="ExternalInput")  # proj_w.T rows
    qkb = nc.dram_tensor("qkb", [1024], F32, kind="ExternalInput")
    vb = nc.dram_tensor("vb", [512], F32, kind="ExternalInput")
    pb = nc.dram_tensor("pb", [D], F32, kind="ExternalInput")
    yT = nc.dram_tensor("yT", [D, T], F32, kind="ExternalOutput")

    Exp = mybir.ActivationFunctionType.Exp
    mult = mybir.AluOpType.mult
    add = mybir.AluOpType.add

    with TileContext(nc) as tc:
        with (
            tc.tile_pool(name="pers", bufs=1) as pers,
            tc.tile_pool(name="small", bufs=1) as small,
        ):
            # ---- persistent tensors ----
            qkts = [
                pers.tile([P, T], BF, tag=f"qkt{i}", name=f"qkt{i}") for i in range(8)
            ]  # feat tiles: 0-3 q, 4-7 k
            V4 = pers.tile([P, 16, 8, HD + 1], BF, tag="v4")  # [t-part, tt, head, 65]
            AT = pers.tile([P, 4, T], BF, tag="at")  # attention out, d-major

            # ---- input loads ----
            ph1_cm = tc.tile_pool(name="ph1", bufs=1)
            ph1 = ph1_cm.__enter__()
            xts = []
            wqks = []
            for dt in range(8):
                t_ = ph1.tile([P, T], BF, tag=f"xt{dt}")
                nc.sync.dma_start(t_[:], xT.ap()[dt * P : (dt + 1) * P, :])
                xts.append(t_)
                w_ = ph1.tile([P, 1024], BF, tag=f"wqk{dt}")
                nc.sync.dma_start(w_[:], wqkT.ap()[dt * P : (dt + 1) * P, :])
                wqks.append(w_)
            wvs = []
            for dt in range(8):
                w_ = ph1.tile([P, 512], BF, tag=f"wv{dt}")
                nc.sync.dma_start(w_[:], wvT.ap()[dt * P : (dt + 1) * P, :])
                wvs.append(w_)
            qkb_sb = small.tile([P, 8], F32, tag="qkb")
            nc.sync.dma_start(qkb_sb[:], qkb.rearrange("(o p) -> p o", p=P))
            vb_sb = small.tile([P, 512], F32, tag="vb")
            nc.sync.dma_start(vb_sb[:], vb.ap()[None, :].to_broadcast((P, 512)))
            pb_sb = small.tile([P, 8], F32, tag="pb")
            nc.sync.dma_start(pb_sb[:], pb.rearrange("(o p) -> p o", p=P))
            # ones columns of V_aug
            nc.vector.memset(V4[:, :, :, HD : HD + 1], 1.0)

            with tc.tile_pool(name="ph1p", bufs=2, space="PSUM") as ph1p:
                # ---- QK^T projection: psum[feat, t] ----
                for ft in range(8):
                    for tcc in range(2):
                        pq = ph1p.tile([P, 1024], F32, tag="pq")
                        for half in range(2):
                            col = tcc * 1024 + half * 512
                            for dt in range(8):
                                nc.tensor.matmul(
                                    pq[:, half * 512 : half * 512 + 512],
                                    lhsT=wqks[dt][:, ft * P : (ft + 1) * P],
                                    rhs=xts[dt][:, col : col + 512],
                                    start=(dt == 0),
                                    stop=(dt == 7),
                                )
                        nc.vector.tensor_scalar_add(
                            qkts[ft][:, tcc * 1024 : (tcc + 1) * 1024],
                            pq[:],
                            qkb_sb[:, ft : ft + 1],
                        )

                # ---- V: psum[t, vfeat] ----
                for tt in range(16):
                    pv = ph1p.tile([P, 512], F32, tag="pv")
                    for dt in range(8):
                        nc.tensor.matmul(
                            pv[:],
                            lhsT=xts[dt][:, tt * P : (tt + 1) * P],
                            rhs=wvs[dt][:],
                            start=(dt == 0),
                            stop=(dt == 7),
                        )
                    nc.vector.tensor_tensor(
                        V4[:, tt, :, 0:HD],
                        pv.rearrange("p (h e) -> p h e", e=HD),
                        vb_sb.rearrange("p (h e) -> p h e", e=HD),
                        mult if False else add,
                    )

            ph1_cm.__exit__(None, None, None)

            # ---- attention ----
            with (
                tc.tile_pool(name="attn", bufs=4) as attn,
                tc.tile_pool(name="norm", bufs=4) as norm,
                tc.tile_pool(name="spool", bufs=2, space="PSUM") as spool,
                tc.tile_pool(name="opool", bufs=2, space="PSUM") as opool,
            ):
                for hp in range(4):  # head pair: heads 2hp (parity 0), 2hp+1 (parity 1)
                    for qc in range(4):  # 512-wide query chunks
                        qcol = qc * 512
                        e0 = attn.tile([P, 16, 512], BF, tag="e")
                        e1 = attn.tile([P, 16, 512], BF, tag="e")
                        es = (e0, e1)
                        pss = [None, None]
                        for kt in range(16):
                            ps0 = spool.tile([P, 512], F32, tag="ps0")
                            ps1 = spool.tile([P, 512], F32, tag="ps1")
                            pss = (ps0, ps1)
                            for par in range(2):
                                rows = slice(par * 64, par * 64 + 64)
                                nc.tensor.matmul(
                                    pss[par][:],
                                    lhsT=qkts[4 + hp][rows, kt * P : (kt + 1) * P],
                                    rhs=qkts[hp][rows, qcol : qcol + 512],
                                    start=True,
                                    stop=True,
                                )
                            for par in range(2):
                                nc.scalar.activation(
                                    es[par][:, kt, :], pss[par][:], Exp, scale=SCALE
                                )
                        pos = [
                            opool.tile([P, 512], F32, tag=f"po{par}", name=f"po{par}")
                            for par in range(2)
                        ]
                        for par in range(2):
                            for kt in range(16):
                                nc.tensor.matmul(
                                    pos[par][0 : HD + 1, :],
                                    lhsT=V4[:, kt, 2 * hp + par, :],
                                    rhs=es[par][:, kt, :],
                                    start=(kt == 0),
                                    stop=(kt == 15),
                                )
                        for par in range(2):
                            po = pos[par]
                            rrow = norm.tile([1, 512], F32, tag="rrow")
                            nc.vector.reciprocal(rrow[:], po[HD : HD + 1, :])
                            rb = norm.tile([64, 512], F32, tag="rb")
                            nc.gpsimd.partition_broadcast(rb[:], rrow[:])
                            nc.vector.tensor_tensor(
                                AT[par * 64 : par * 64 + 64, hp, qcol : qcol + 512],
                                po[0:HD, :],
                                rb[:],
                                mult,
                            )

            # ---- projection: yT[e, t] = wpT.T @ AT ----
            with (
                tc.tile_pool(name="ph3", bufs=1) as ph3,
                tc.tile_pool(name="outp", bufs=3) as outp,
                tc.tile_pool(name="ph3p", bufs=2, space="PSUM") as ph3p,
            ):
                wps = []
                for dt in range(4):
                    w_ = ph3.tile([P, D], BF, tag=f"wp{dt}")
                    nc.sync.dma_start(w_[:], wpT.ap()[dt * P : (dt + 1) * P, :])
                    wps.append(w_)
                for et in range(8):
                    for tcc in range(2):
                        pp = ph3p.tile([P, 1024], F32, tag="pp")
                        for half in range(2):
                            col = tcc * 1024 + half * 512
                            for dt in range(4):
                                nc.tensor.matmul(
                                    pp[:, half * 512 : half * 512 + 512],
                                    lhsT=wps[dt][:, et * P : (et + 1) * P],
                                    rhs=AT[:, dt, col : col + 512],
                                    start=(dt == 0),
                                    stop=(dt == 3),
                                )
                        ob = outp.tile([P, 1024], F32, tag="ob")
                        nc.vector.tensor_scalar_add(ob[:], pp[:], pb_sb[:, et : et + 1])
                        nc.sync.dma_start(
                            yT.ap()[et * P : (et + 1) * P, tcc * 1024 : (tcc + 1) * 1024],
                            ob[:],
                        )

    nc.compile()
    return nc


def kernel(x, qkv_w, qkv_b, proj_w, proj_b):
    global _built, LAST_RESULT
    x = np.asarray(x, np.float32)
    qkv_w = np.asarray(qkv_w, np.float32)
    qkv_b = np.asarray(qkv_b, np.float32)
    proj_w = np.asarray(proj_w, np.float32)
    proj_b = np.asarray(proj_b, np.float32)

    if _built is None:
        _built = _build()
    nc = _built

    in_maps = []
    for c in range(8):
        b, j = divmod(c, 2)
        s = j * 512
        wqkT = np.concatenate([qkv_w[s : s + 512], qkv_w[1024 + s : 1024 + s + 512]]).T
        in_maps.append(
            {
                "xT": np.ascontiguousarray(x[b].T).astype(NBF),
                "wqkT": np.ascontiguousarray(wqkT).astype(NBF),
                "wvT": np.ascontiguousarray(qkv_w[2048 + s : 2048 + s + 512].T).astype(NBF),
                "wpT": np.ascontiguousarray(proj_w[:, s : s + 512].T).astype(NBF),
                "qkb": np.concatenate([qkv_b[s : s + 512], qkv_b[1024 + s : 1024 + s + 512]]),
                "vb": np.ascontiguousarray(qkv_b[2048 + s : 2048 + s + 512]),
                "pb": proj_b if j == 0 else np.zeros_like(proj_b),
            }
        )

    trace = os.environ.get("BASS_TRACE") == "1"
    res = run_bass_kernel_spmd(nc, in_maps, core_ids=list(range(8)), trace=trace)
    LAST_RESULT = res

    out = np.empty((B, T, D), np.float32)
    for b in range(B):
        out[b] = (res.results[2 * b]["yT"] + res.results[2 * b + 1]["yT"]).T
    return out
